# revision 3
# baseline (speedup 1.0000x reference)
"""DyGNN streaming-interaction kernel for Trainium2 (8 NeuronCores, Bass/Tile).

Strategy
--------
The reference is a sequential scan over S=2048 events touching rows of five
[N=100000, 128] node-state tables.  The output is only the PRE-update node
representation gathered at each event, so an event's update math matters only
if a LATER event reads one of its two nodes.  With random indices that is a
small set ("relevant" events, ~82 for the expected data) with a very shallow
dependency depth (2 levels).

Host side (index math only): find relevant events, batch them into dependency
levels, compute operand provenance, and route the 2*S output-row gathers to
the core owning each node (node_rep is sharded row-wise across the 8 cores).

Device side (single SPMD program, per-core data):
  * each core indirect-DMA-gathers its share of output rows from its
    node_rep shard and writes them to a compact output buffer;
  * the relevant-event recurrence (edge updaters + time-decayed LSTMs +
    combiner) runs as batched matmuls in a transposed [feature, event]
    layout, one batch per dependency level (replicated on all cores - it is
    tiny - core 0's result is used).

Host side assembles the [2, S, D] output from the per-core gather buffers
plus the computed representations for the few "patched" slots.
"""

import os
import numpy as np

_NCORES = 8
_D = 128          # embedding dim == partition count
_MAXB = 512       # max events per device batch (one PSUM bank of fp32)
_MAX_LEVELS = 64  # beyond this (adversarial chains) use the host fallback
_W_DECAY = 1.0

# operand slab order in the per-level seed tensor
# RH=rep[h]  RT=rep[t]  CH=cell_head[h]  HH=hidden_head[h]  CT=cell_tail[t]
# HT=hidden_tail[t]  XHT=hidden_tail[h]  XHH=hidden_head[t]  DTH/DTT=delta-t
_N_OPS = 10
_OP_RH, _OP_RT, _OP_CH, _OP_HH, _OP_CT, _OP_HT, _OP_XHT, _OP_XHH, _OP_DTH, _OP_DTT = range(10)

_KINDS = ("CHN", "HHN", "CTN", "HTN", "NRH", "NRT")

_cache = {}
last_result = None  # BassKernelResults of the most recent device run


def _preprocess(heads, tails, times):
    """Pure index/time analysis.  Returns None if the dependency structure is
    too deep for the compiled-levels approach (host fallback handles it)."""
    S = heads.shape[0]

    # -- pass 1 (backward): does any later event touch this event's nodes? --
    touched_later = np.zeros(S, dtype=bool)
    seen = {}
    for i in range(S - 1, -1, -1):
        h = int(heads[i]); t = int(tails[i])
        touched_later[i] = (h in seen) or (t in seen)
        seen[h] = True; seen[t] = True
    rel = [i for i in range(S) if touched_later[i]]

    # -- pass 2: assign dependency levels (width-capped at _MAXB) --
    level_events = []
    last_level = {}
    pos_of = {}
    for i in rel:
        h = int(heads[i]); t = int(tails[i])
        lv = max(last_level.get(h, 0), last_level.get(t, 0)) + 1
        while lv - 1 < len(level_events) and len(level_events[lv - 1]) >= _MAXB:
            lv += 1
        if lv > _MAX_LEVELS:
            return None
        while len(level_events) < lv:
            level_events.append([])
        pos_of[i] = (lv - 1, len(level_events[lv - 1]))
        level_events[lv - 1].append(i)
        last_level[h] = lv; last_level[t] = lv

    Bs = [len(evs) for evs in level_events]
    # comp column layout: per level, NRH columns then NRT columns
    off = [0]
    for b in Bs:
        off.append(off[-1] + 2 * b)
    Ctot = off[-1]

    # -- pass 3 (forward over relevant events): operand provenance --
    # copies: per level, list of (op_idx, dst_col, src_level, src_kind, src_col)
    copies = [[] for _ in Bs]
    seed_fill = [[] for _ in Bs]  # (op_idx, col, node) -> filled from tables later
    dt_fill = [[] for _ in Bs]    # (col, dt_h, dt_t)
    lastw = {"rep": {}, "ch": {}, "hh": {}, "ct": {}, "ht": {}}
    last_time = {}
    for i in rel:
        h = int(heads[i]); t = int(tails[i]); tm = np.float32(times[i])
        lv, p = pos_of[i]

        def src_or_seed(op_idx, src, node):
            if src is not None:
                slv, sp, skind = src
                copies[lv].append((op_idx, p, slv, skind, sp))
            else:
                seed_fill[lv].append((op_idx, p, node))

        src_or_seed(_OP_RH, lastw["rep"].get(h), ("node_rep", h))
        src_or_seed(_OP_RT, lastw["rep"].get(t), ("node_rep", t))
        src_or_seed(_OP_CH, lastw["ch"].get(h), ("cell_head", h))
        src_or_seed(_OP_HH, lastw["hh"].get(h), ("hidden_head", h))
        src_or_seed(_OP_CT, lastw["ct"].get(t), ("cell_tail", t))
        src_or_seed(_OP_HT, lastw["ht"].get(t), ("hidden_tail", t))
        src_or_seed(_OP_XHT, lastw["ht"].get(h), ("hidden_tail", h))
        src_or_seed(_OP_XHH, lastw["hh"].get(t), ("hidden_head", t))
        dt_h = np.float32(tm - np.float32(last_time.get(h, 0.0)))
        dt_t = np.float32(tm - np.float32(last_time.get(t, 0.0)))
        dt_fill[lv].append((p, dt_h, dt_t))

        # writes: head role first, then tail role (tail wins on self-loops)
        lastw["rep"][h] = (lv, p, "NRH")
        lastw["rep"][t] = (lv, p, "NRT")
        lastw["ch"][h] = (lv, p, "CHN")
        lastw["hh"][h] = (lv, p, "HHN")
        lastw["ct"][t] = (lv, p, "CTN")
        lastw["ht"][t] = (lv, p, "HTN")
        last_time[h] = tm; last_time[t] = tm

    return {
        "rel": rel,
        "touched_later": touched_later,
        "pos_of": pos_of,
        "Bs": Bs,
        "off": off,
        "Ctot": Ctot,
        "copies": copies,
        "seed_fill": seed_fill,
        "dt_fill": dt_fill,
    }


def _route_outputs(heads, tails, plan, N):
    """Route each of the 2*S output slots to either a per-core gather or a
    computed-rep column.  Returns routing tables + padded local index arrays."""
    S = heads.shape[0]
    Bs, off, pos_of = plan["Bs"], plan["off"], plan["pos_of"]
    touched_later = plan["touched_later"]
    shard = -(-N // _NCORES)  # ceil

    gl_idx = [[] for _ in range(_NCORES)]
    gl_slot = [[] for _ in range(_NCORES)]
    comp_slots, comp_cols = [], []
    lastw_rep_col = {}
    for i in range(S):
        h = int(heads[i]); t = int(tails[i])
        for role, n in ((0, h), (1, t)):
            slot = role * S + i
            cc = lastw_rep_col.get(n)
            if cc is not None:
                comp_slots.append(slot); comp_cols.append(cc)
            else:
                k = n // shard
                gl_idx[k].append(n - k * shard)
                gl_slot[k].append(slot)
        if touched_later[i]:
            lv, p = pos_of[i]
            lastw_rep_col[h] = off[lv] + p           # NRH column
            lastw_rep_col[t] = off[lv] + Bs[lv] + p  # NRT column

    max_load = max(len(x) for x in gl_idx)
    G = max(1, -(-max_load // 128))
    n_pad = G * 128
    oidx = []
    for k in range(_NCORES):
        a = np.zeros(n_pad, dtype=np.int32)
        a[: len(gl_idx[k])] = gl_idx[k]
        # slot g*128+p maps to SBUF [p, g]
        oidx.append(np.ascontiguousarray(a.reshape(G, 128).T))
    return {
        "shard": shard,
        "G": G,
        "n_pad": n_pad,
        "oidx": oidx,
        "gl_slot": gl_slot,
        "comp_slots": np.array(comp_slots, dtype=np.int64),
        "comp_cols": np.array(comp_cols, dtype=np.int64),
    }


def _build_program(shard, G, Bs, copies, Ctot):
    from contextlib import ExitStack

    import concourse.bacc as bacc
    import concourse.bass as bass
    import concourse.tile as tile
    from concourse import mybir

    f32 = mybir.dt.float32
    i32 = mybir.dt.int32
    AFT = mybir.ActivationFunctionType

    nc = bacc.Bacc(
        "TRN2",
        debug=False,
        enable_asserts=False,
        target_bir_lowering=False,
        num_devices=_NCORES,
    )

    rep = nc.dram_tensor("rep", [shard, _D], f32, kind="ExternalInput").ap()
    oidx = nc.dram_tensor("oidx", [128, G], i32, kind="ExternalInput").ap()
    out_gath = nc.dram_tensor("out_gath", [G * 128, _D], f32, kind="ExternalOutput").ap()

    L = len(Bs)
    w_shapes = {
        "Weh1": 128, "Weh2": 128, "Wet1": 128, "Wet2": 128,
        "Wdh": 128, "Wdt": 128, "Wc1": 128, "Wc2": 128,
        "Wxh": 512, "Whh": 512, "Wxt": 512, "Wht": 512,
    }
    b_shapes = {"beh": 1, "bet": 1, "bdh": 1, "bdt": 1, "bh4": 4, "bt4": 4}
    w_dram = {}
    seeds_dram = []
    comp = None
    if L:
        for name, cols in w_shapes.items():
            w_dram[name] = nc.dram_tensor(name, [128, cols], f32, kind="ExternalInput").ap()
        for name, cols in b_shapes.items():
            w_dram[name] = nc.dram_tensor(name, [128, cols], f32, kind="ExternalInput").ap()
        for l, B in enumerate(Bs):
            seeds_dram.append(
                nc.dram_tensor(f"seeds{l}", [_N_OPS, 128, B], f32, kind="ExternalInput").ap()
            )
        comp = nc.dram_tensor("comp", [128, Ctot], f32, kind="ExternalOutput").ap()

    with tile.TileContext(nc) as tc, ExitStack() as ctx:
        wp = ctx.enter_context(tc.tile_pool(name="w", bufs=1))
        lp = ctx.enter_context(tc.tile_pool(name="lv", bufs=1))
        tp = ctx.enter_context(tc.tile_pool(name="tmp", bufs=3))
        pp = ctx.enter_context(tc.tile_pool(name="ps", bufs=8, space="PSUM"))
        gp = ctx.enter_context(tc.tile_pool(name="g", bufs=4))

        # ---- output gather phase (bulk of the memory traffic) ----
        idx_sb = gp.tile([128, G], i32, tag="idx", bufs=1)
        nc.sync.dma_start(idx_sb[:], oidx[:])
        for g in range(G):
            gt = gp.tile([128, _D], f32, tag="gath")
            nc.gpsimd.indirect_dma_start(
                out=gt[:],
                out_offset=None,
                in_=rep[:],
                in_offset=bass.IndirectOffsetOnAxis(ap=idx_sb[:, g : g + 1], axis=0),
            )
            nc.sync.dma_start(out_gath[g * 128 : (g + 1) * 128, :], gt[:])

        # ---- relevant-event recurrence (transposed [feature, event] layout) ----
        if L:
            w_sb = {}
            for name in list(w_shapes) + list(b_shapes):
                cols = w_shapes.get(name) or b_shapes[name]
                t = wp.tile([128, cols], f32, tag=name)
                nc.sync.dma_start(t[:], w_dram[name][:])
                w_sb[name] = t

            results = []  # per level: dict kind -> tile
            for l, B in enumerate(Bs):
                ops = []
                for o in range(_N_OPS):
                    t = lp.tile([128, B], f32, tag=f"op{l}_{o}")
                    nc.sync.dma_start(t[:], seeds_dram[l][o, :, :])
                    ops.append(t)
                for (op_i, c_dst, slv, skind, c_src) in copies[l]:
                    nc.vector.tensor_copy(
                        ops[op_i][:, c_dst : c_dst + 1],
                        results[slv][skind][:, c_src : c_src + 1],
                    )
                RH, RT, CH, HH, CT, HT, XHT, XHH, DTH, DTT = ops

                def mm(w1, r1, w2=None, r2=None, q=None):
                    ps = pp.tile([128, B], f32, tag="ps")
                    sl = slice(None) if q is None else slice(q * 128, (q + 1) * 128)
                    nc.tensor.matmul(ps[:], w_sb[w1][:, sl], r1[:], start=True, stop=w2 is None)
                    if w2 is not None:
                        nc.tensor.matmul(ps[:], w_sb[w2][:, sl], r2[:], start=False, stop=True)
                    return ps

                def act(src, func, bias=0.0, scale=1.0, tag="t"):
                    o = tp.tile([128, B], f32, tag=tag)
                    nc.scalar.activation(o[:], src[:], func, bias=bias, scale=scale)
                    return o

                def vop(fn, a, b, tag="v"):
                    o = tp.tile([128, B], f32, tag=tag)
                    fn(o[:], a[:], b[:])
                    return o

                res = {}
                for side, (Cx, Hx, DT, We1, We2, be, Wd, bd, Wx, Wh, b4,
                           ck, hk) in enumerate([
                    (CH, HH, DTH, "Weh1", "Weh2", "beh", "Wdh", "bdh",
                     "Wxh", "Whh", "bh4", "CHN", "HHN"),
                    (CT, HT, DTT, "Wet1", "Wet2", "bet", "Wdt", "bdt",
                     "Wxt", "Wht", "bt4", "CTN", "HTN"),
                ]):
                    dec = act(DT, AFT.Exp, scale=-_W_DECAY, tag=f"dec{side}")
                    edge = act(mm(We1, RH, We2, RT), AFT.Tanh,
                               bias=w_sb[be][:, 0:1], tag=f"edge{side}")
                    cs = act(mm(Wd, Cx), AFT.Tanh, bias=w_sb[bd][:, 0:1], tag=f"cs{side}")
                    csdec = vop(nc.vector.tensor_mul, cs, dec, tag=f"csd{side}")
                    cmcs = vop(nc.vector.tensor_sub, Cx, cs, tag=f"cmc{side}")
                    cadj = vop(nc.vector.tensor_add, cmcs, csdec, tag=f"cad{side}")
                    gates = []
                    for q, gf in enumerate([AFT.Sigmoid, AFT.Sigmoid, AFT.Sigmoid, AFT.Tanh]):
                        ps = mm(Wx, edge, Wh, Hx, q=q)
                        gates.append(act(ps, gf, bias=w_sb[b4][:, q : q + 1], tag=f"g{side}{q}"))
                    gi, gf_, go, gg = gates
                    fc = vop(nc.vector.tensor_mul, gf_, cadj, tag=f"fc{side}")
                    ig = vop(nc.vector.tensor_mul, gi, gg, tag=f"ig{side}")
                    c_new = lp.tile([128, B], f32, tag=f"res{l}_{ck}")
                    nc.vector.tensor_add(c_new[:], fc[:], ig[:])
                    tc_ = act(c_new, AFT.Tanh, tag=f"tc{side}")
                    h_new = lp.tile([128, B], f32, tag=f"res{l}_{hk}")
                    nc.vector.tensor_mul(h_new[:], go[:], tc_[:])
                    res[ck] = c_new
                    res[hk] = h_new

                # combiner uses pre-step cross hiddens XHT / XHH
                nrh = lp.tile([128, B], f32, tag=f"res{l}_NRH")
                nc.scalar.activation(nrh[:], mm("Wc1", res["HHN"], "Wc2", XHT)[:], AFT.Tanh)
                nrt = lp.tile([128, B], f32, tag=f"res{l}_NRT")
                nc.scalar.activation(nrt[:], mm("Wc1", XHH, "Wc2", res["HTN"])[:], AFT.Tanh)
                res["NRH"] = nrh
                res["NRT"] = nrt
                results.append(res)

                o0 = sum(2 * b for b in Bs[:l])
                nc.sync.dma_start(comp[:, o0 : o0 + B], nrh[:])
                nc.sync.dma_start(comp[:, o0 + B : o0 + 2 * B], nrt[:])

    nc.compile()
    return nc


def _pack_weight_inputs(inputs):
    f32 = np.float32
    w = {}
    for name in ("Weh1", "Weh2", "Wet1", "Wet2", "Wdh", "Wdt", "Wc1", "Wc2",
                 "Wxh", "Whh", "Wxt", "Wht"):
        w[name] = np.ascontiguousarray(inputs[name], dtype=f32)
    for name, src in (("beh", "beh"), ("bet", "bet"), ("bdh", "bdh"), ("bdt", "bdt")):
        w[name] = np.ascontiguousarray(inputs[src], dtype=f32).reshape(128, 1)
    w["bh4"] = np.ascontiguousarray(np.asarray(inputs["bh"], f32).reshape(4, 128).T)
    w["bt4"] = np.ascontiguousarray(np.asarray(inputs["bt"], f32).reshape(4, 128).T)
    return w


def _numpy_fallback(heads, tails, times, node_rep, cell_head, hidden_head,
                    cell_tail, hidden_tail, Weh1, Weh2, beh, Wet1, Wet2, bet,
                    Wxh, Whh, bh, Wdh, bdh, Wxt, Wht, bt, Wdt, bdt, Wc1, Wc2):
    """Exact float32 reference semantics; safety net for pathological inputs."""
    f32 = np.float32
    S = heads.shape[0]
    D = node_rep.shape[1]
    rep = np.array(node_rep, f32); ch = np.array(cell_head, f32)
    hh = np.array(hidden_head, f32); ct = np.array(cell_tail, f32)
    ht = np.array(hidden_tail, f32)
    rt = np.zeros(node_rep.shape[0], f32)
    out = np.zeros((2, S, D), f32)

    def sig(x):
        return f32(1.0) / (f32(1.0) + np.exp(-x, dtype=f32))

    def tlstm(x, c, h, dec, Wx, Wh, b, Wd, bd):
        cs = np.tanh(c @ Wd + bd, dtype=f32)
        c_adj = c - cs + cs * dec
        z = x @ Wx + h @ Wh + b
        i, f, o, g = np.split(z, 4)
        i = sig(i); f = sig(f); o = sig(o); g = np.tanh(g, dtype=f32)
        c_new = f * c_adj + i * g
        return c_new, o * np.tanh(c_new, dtype=f32)

    for j in range(S):
        h_i = int(heads[j]); t_i = int(tails[j]); tm = f32(times[j])
        rep_h = rep[h_i].copy(); rep_t = rep[t_i].copy()
        out[0, j] = rep_h; out[1, j] = rep_t
        dec_h = np.exp(f32(-_W_DECAY) * (tm - rt[h_i]), dtype=f32)
        dec_t = np.exp(f32(-_W_DECAY) * (tm - rt[t_i]), dtype=f32)
        edge_h = np.tanh(rep_h @ Weh1 + rep_t @ Weh2 + beh, dtype=f32)
        edge_t = np.tanh(rep_h @ Wet1 + rep_t @ Wet2 + bet, dtype=f32)
        c_h, h_h = tlstm(edge_h, ch[h_i], hh[h_i], dec_h, Wxh, Whh, bh, Wdh, bdh)
        c_t, h_t = tlstm(edge_t, ct[t_i], ht[t_i], dec_t, Wxt, Wht, bt, Wdt, bdt)
        new_rep_h = np.tanh(h_h @ Wc1 + ht[h_i] @ Wc2, dtype=f32)
        new_rep_t = np.tanh(hh[t_i] @ Wc1 + h_t @ Wc2, dtype=f32)
        rep[h_i] = new_rep_h; rep[t_i] = new_rep_t
        ch[h_i] = c_h; hh[h_i] = h_h; ct[t_i] = c_t; ht[t_i] = h_t
        rt[h_i] = tm; rt[t_i] = tm
    return out


def kernel(**inputs):
    global last_result
    heads = np.asarray(inputs["heads"]).astype(np.int64)
    tails = np.asarray(inputs["tails"]).astype(np.int64)
    times = np.asarray(inputs["times"], dtype=np.float32)
    node_rep = np.asarray(inputs["node_rep"], dtype=np.float32)
    N = node_rep.shape[0]
    S = heads.shape[0]

    plan = _preprocess(heads, tails, times)
    if plan is None:
        return _numpy_fallback(
            heads, tails, times,
            *[np.asarray(inputs[k], np.float32) for k in (
                "node_rep", "cell_head", "hidden_head", "cell_tail", "hidden_tail",
                "Weh1", "Weh2", "beh", "Wet1", "Wet2", "bet",
                "Wxh", "Whh", "bh", "Wdh", "bdh", "Wxt", "Wht", "bt", "Wdt", "bdt",
                "Wc1", "Wc2")],
        )

    routing = _route_outputs(heads, tails, plan, N)
    shard, G, n_pad = routing["shard"], routing["G"], routing["n_pad"]
    Bs, Ctot = plan["Bs"], plan["Ctot"]
    L = len(Bs)

    sig = (shard, G, tuple(Bs),
           tuple(tuple(c) for lc in plan["copies"] for c in lc), Ctot)
    nc = _cache.get(sig)
    if nc is None:
        nc = _build_program(shard, G, Bs, plan["copies"], Ctot)
        _cache[sig] = nc

    # per-level seed tensors from the state tables (+ delta-t slabs)
    tables = {k: np.asarray(inputs[k], np.float32) for k in (
        "node_rep", "cell_head", "hidden_head", "cell_tail", "hidden_tail")}
    seeds = [np.zeros((_N_OPS, 128, B), np.float32) for B in Bs]
    for l in range(L):
        for (op_idx, col, (tab, node)) in plan["seed_fill"][l]:
            seeds[l][op_idx, :, col] = tables[tab][node]
        for (col, dt_h, dt_t) in plan["dt_fill"][l]:
            seeds[l][_OP_DTH, :, col] = dt_h
            seeds[l][_OP_DTT, :, col] = dt_t

    w_in = _pack_weight_inputs(inputs) if L else {}

    pad_rows = shard * _NCORES - N
    rep_padded = node_rep if pad_rows == 0 else np.vstack(
        [node_rep, np.zeros((pad_rows, _D), np.float32)])
    in_maps = []
    for k in range(_NCORES):
        m = {
            "rep": np.ascontiguousarray(rep_padded[k * shard : (k + 1) * shard]),
            "oidx": routing["oidx"][k],
        }
        if L:
            m.update(w_in)
            for l in range(L):
                m[f"seeds{l}"] = seeds[l]
        in_maps.append(m)

    from concourse import bass_utils
    res = bass_utils.run_bass_kernel_spmd(nc, in_maps, core_ids=list(range(_NCORES)))
    last_result = res

    out_flat = np.zeros((2 * S, _D), np.float32)
    for k in range(_NCORES):
        slots = routing["gl_slot"][k]
        if slots:
            out_flat[np.asarray(slots)] = res.results[k]["out_gath"][: len(slots)]
    if len(routing["comp_slots"]):
        comp_v = res.results[0]["comp"]
        out_flat[routing["comp_slots"]] = comp_v[:, routing["comp_cols"]].T
    return out_flat.reshape(2, S, _D)


# revision 6
# speedup vs baseline: 1.3694x; 1.3694x over previous
"""DyGNN streaming-interaction kernel for Trainium2 (8 NeuronCores, Bass/Tile).

Strategy
--------
The reference is a sequential scan over S=2048 events touching rows of five
[N=100000, 128] node-state tables.  The output is only the PRE-update node
representation gathered at each event, so an event's update math matters only
if a LATER event reads one of its two nodes.  With random indices that is a
small set ("relevant" events, ~82 for the expected data) with a very shallow
dependency depth (~2 levels).

Host side (index math only): find relevant events, batch them into dependency
levels, compute operand provenance, and route the 2*S output-row gathers to
the core owning each node (node_rep is sharded row-wise across the 8 cores).

Device side (single SPMD program, per-core data):
  * each core gathers its share of output rows from its node_rep shard with
    one multi-row indirect DMA and writes them out contiguously;
  * the relevant-event recurrence (edge updaters + time-decayed LSTMs +
    combiner) runs as batched fp32 matmuls in a transposed
    [feature, head-events | tail-events] layout, one batch per dependency
    level (replicated on all cores - it is tiny - core 0's result is used).
    Sigmoid is computed as 0.5 + 0.5*tanh(x/2) so the whole kernel uses a
    single ACT table set (exp+tanh) - no table-switch stalls.

Host side assembles the [2, S, D] output from the per-core gather buffers
plus the computed representations for the few "patched" slots.
"""

import numpy as np

_NCORES = 8
_D = 128          # embedding dim == partition count
_MAXB = 256       # max events per device batch ([128, 2B] fits one PSUM bank)
_MAX_LEVELS = 64  # beyond this (adversarial chains) use the host fallback
_W_DECAY = 1.0

# operand order inside the packed per-level seed tile [128, 10*B]:
# RH RT | CH CT | HH HT | XHT XHH | DTH DTT   (each block is B columns)
_OP_RH, _OP_RT, _OP_CH, _OP_CT, _OP_HH, _OP_HT, _OP_XHT, _OP_XHH, _OP_DTH, _OP_DTT = range(10)
_N_OPS = 10

_cache = {}
last_result = None  # BassKernelResults of the most recent device run


def _preprocess(heads, tails, times):
    """Pure index/time analysis.  Returns None if the dependency structure is
    too deep for the compiled-levels approach (host fallback handles it)."""
    S = heads.shape[0]

    # -- pass 1 (backward): does any later event touch this event's nodes? --
    touched_later = np.zeros(S, dtype=bool)
    seen = {}
    for i in range(S - 1, -1, -1):
        h = int(heads[i]); t = int(tails[i])
        touched_later[i] = (h in seen) or (t in seen)
        seen[h] = True; seen[t] = True
    rel = [i for i in range(S) if touched_later[i]]

    # -- pass 2: assign dependency levels (width-capped at _MAXB) --
    level_events = []
    last_level = {}
    pos_of = {}
    for i in rel:
        h = int(heads[i]); t = int(tails[i])
        lv = max(last_level.get(h, 0), last_level.get(t, 0)) + 1
        while lv - 1 < len(level_events) and len(level_events[lv - 1]) >= _MAXB:
            lv += 1
        if lv > _MAX_LEVELS:
            return None
        while len(level_events) < lv:
            level_events.append([])
        pos_of[i] = (lv - 1, len(level_events[lv - 1]))
        level_events[lv - 1].append(i)
        last_level[h] = lv; last_level[t] = lv

    Bs = [len(evs) for evs in level_events]
    off = [0]
    for b in Bs:
        off.append(off[-1] + 2 * b)
    Ctot = off[-1]

    # -- pass 3 (forward over relevant events): operand provenance --
    # copies: per level, (dst_col_in_seed_tile, src_level, src_tile, src_col)
    # src_tile in {"C", "H", "R"}: packed result tiles [CHN|CTN], [HHN|HTN],
    # [NRH|NRT] of the source level.
    copies = [[] for _ in Bs]
    seed_fill = [[] for _ in Bs]  # (packed_col, table, node)
    dt_fill = [[] for _ in Bs]    # (pos, dt_h, dt_t)
    lastw = {"rep": {}, "ch": {}, "hh": {}, "ct": {}, "ht": {}}
    last_time = {}
    for i in rel:
        h = int(heads[i]); t = int(tails[i]); tm = np.float32(times[i])
        lv, p = pos_of[i]
        B = Bs[lv]

        def src_or_seed(op_idx, src, table, node):
            dst_col = op_idx * B + p
            if src is not None:
                slv, sp, skind = src
                stile = {"CHN": "C", "CTN": "C", "HHN": "H", "HTN": "H",
                         "NRH": "R", "NRT": "R"}[skind]
                s_col = sp if skind in ("CHN", "HHN", "NRH") else Bs[slv] + sp
                copies[lv].append((dst_col, slv, stile, s_col))
            else:
                seed_fill[lv].append((dst_col, table, node))

        src_or_seed(_OP_RH, lastw["rep"].get(h), "node_rep", h)
        src_or_seed(_OP_RT, lastw["rep"].get(t), "node_rep", t)
        src_or_seed(_OP_CH, lastw["ch"].get(h), "cell_head", h)
        src_or_seed(_OP_CT, lastw["ct"].get(t), "cell_tail", t)
        src_or_seed(_OP_HH, lastw["hh"].get(h), "hidden_head", h)
        src_or_seed(_OP_HT, lastw["ht"].get(t), "hidden_tail", t)
        src_or_seed(_OP_XHT, lastw["ht"].get(h), "hidden_tail", h)
        src_or_seed(_OP_XHH, lastw["hh"].get(t), "hidden_head", t)
        dt_h = np.float32(tm - np.float32(last_time.get(h, 0.0)))
        dt_t = np.float32(tm - np.float32(last_time.get(t, 0.0)))
        dt_fill[lv].append((p, dt_h, dt_t))

        # writes: head role first, then tail role (tail wins on self-loops)
        lastw["rep"][h] = (lv, p, "NRH")
        lastw["rep"][t] = (lv, p, "NRT")
        lastw["ch"][h] = (lv, p, "CHN")
        lastw["hh"][h] = (lv, p, "HHN")
        lastw["ct"][t] = (lv, p, "CTN")
        lastw["ht"][t] = (lv, p, "HTN")
        last_time[h] = tm; last_time[t] = tm

    return {
        "touched_later": touched_later,
        "pos_of": pos_of,
        "Bs": Bs,
        "off": off,
        "Ctot": Ctot,
        "copies": copies,
        "seed_fill": seed_fill,
        "dt_fill": dt_fill,
    }


def _route_outputs(heads, tails, plan, N):
    """Route each of the 2*S output slots to either a per-core gather or a
    computed-rep column."""
    S = heads.shape[0]
    Bs, off, pos_of = plan["Bs"], plan["off"], plan["pos_of"]
    touched_later = plan["touched_later"]
    shard = -(-N // _NCORES)  # ceil

    gl_idx = [[] for _ in range(_NCORES)]
    gl_slot = [[] for _ in range(_NCORES)]
    comp_slots, comp_cols = [], []
    lastw_rep_col = {}
    for i in range(S):
        h = int(heads[i]); t = int(tails[i])
        for role, n in ((0, h), (1, t)):
            slot = role * S + i
            cc = lastw_rep_col.get(n)
            if cc is not None:
                comp_slots.append(slot); comp_cols.append(cc)
            else:
                k = n // shard
                gl_idx[k].append(n - k * shard)
                gl_slot[k].append(slot)
        if touched_later[i]:
            lv, p = pos_of[i]
            lastw_rep_col[h] = off[lv] + p           # NRH column
            lastw_rep_col[t] = off[lv] + Bs[lv] + p  # NRT column

    max_load = max(len(x) for x in gl_idx)
    G = max(1, -(-max_load // 128))
    n_pad = G * 128
    oidx = []
    for k in range(_NCORES):
        a = np.zeros(n_pad, dtype=np.int32)
        a[: len(gl_idx[k])] = gl_idx[k]
        # gathered row g*128+p comes from SBUF [p, g*128:(g+1)*128]
        oidx.append(np.ascontiguousarray(a.reshape(G, 128).T))
    return {
        "shard": shard,
        "G": G,
        "n_pad": n_pad,
        "oidx": oidx,
        "gl_slot": gl_slot,
        "comp_slots": np.array(comp_slots, dtype=np.int64),
        "comp_cols": np.array(comp_cols, dtype=np.int64),
    }


# packed weight layouts (name -> column width); A feeds the first matmuls
_WPACK_A = (("Weh1", 128), ("Weh2", 128), ("Wet1", 128), ("Wet2", 128),
            ("Wdh", 128), ("Wdt", 128))
_WPACK_B = (("Wxh", 512), ("Whh", 512), ("Wxt", 512), ("Wht", 512),
            ("Wc1", 128), ("Wc2", 128))
_BPACK = (("beh", 1), ("bet", 1), ("bdh", 1), ("bdt", 1),
          ("bh4h", 4), ("bt4h", 4), ("bh4", 4), ("bt4", 4))


def _wcols(pack):
    offs, o = {}, 0
    for name, w in pack:
        offs[name] = (o, w)
        o += w
    return offs, o


def _build_program(shard, G, Bs, copies, Ctot, has_bias):
    from contextlib import ExitStack

    import concourse.bacc as bacc
    import concourse.bass as bass
    import concourse.tile as tile
    from concourse import mybir

    f32 = mybir.dt.float32
    i32 = mybir.dt.int32
    AFT = mybir.ActivationFunctionType

    nc = bacc.Bacc(
        "TRN2",
        debug=False,
        enable_asserts=False,
        target_bir_lowering=False,
        num_devices=_NCORES,
    )

    rep = nc.dram_tensor("rep", [shard, _D], f32, kind="ExternalInput").ap()
    oidx = nc.dram_tensor("oidx", [128, G], i32, kind="ExternalInput").ap()
    out_gath = nc.dram_tensor("out_gath", [128, G * _D], f32, kind="ExternalOutput").ap()

    L = len(Bs)
    offsA, WA = _wcols(_WPACK_A)
    offsB, WB = _wcols(_WPACK_B)
    offsBias, WBias = _wcols(_BPACK)
    seeds_dram = []
    comp = None
    if L:
        wpackA = nc.dram_tensor("wpackA", [128, WA], f32, kind="ExternalInput").ap()
        wpackB = nc.dram_tensor("wpackB", [128, WB], f32, kind="ExternalInput").ap()
        if has_bias:
            bpack = nc.dram_tensor("bpack", [128, WBias], f32, kind="ExternalInput").ap()
        for l, B in enumerate(Bs):
            seeds_dram.append(
                nc.dram_tensor(f"seeds{l}", [128, _N_OPS * B], f32, kind="ExternalInput").ap()
            )
        comp = nc.dram_tensor("comp", [128, Ctot], f32, kind="ExternalOutput").ap()

    with tile.TileContext(nc) as tc, ExitStack() as ctx:
        wp = ctx.enter_context(tc.tile_pool(name="w", bufs=1))
        lp = ctx.enter_context(tc.tile_pool(name="lv", bufs=1))
        tp = ctx.enter_context(tc.tile_pool(name="tmp", bufs=3))
        pp = ctx.enter_context(tc.tile_pool(name="ps", bufs=8, space="PSUM"))
        gp = ctx.enter_context(tc.tile_pool(name="g", bufs=1))

        # ---- stage weight/seed DMAs on the ACT HWDGE ring (parallel to
        # ---- the gather's sync ring), seeds first: they start the chain
        if L:
            sd_tiles = []
            for l, B in enumerate(Bs):
                t = lp.tile([128, _N_OPS * B], f32, tag=f"sd{l}", name=f"sd{l}")
                nc.scalar.dma_start(t[:], seeds_dram[l][:])
                sd_tiles.append(t)
            wA = wp.tile([128, WA], f32, tag="wA", name="wA")
            nc.scalar.dma_start(wA[:], wpackA[:])
            wB = wp.tile([128, WB], f32, tag="wB", name="wB")
            nc.scalar.dma_start(wB[:], wpackB[:])
            if has_bias:
                bt_ = wp.tile([128, WBias], f32, tag="bias", name="biasT")
                nc.scalar.dma_start(bt_[:], bpack[:])

            def w(name):
                if name in offsA:
                    o, wd = offsA[name]
                    return wA[:, o : o + wd]
                o, wd = offsB[name]
                return wB[:, o : o + wd]

            def bias_ap(name, col=0):
                o, _ = offsBias[name]
                return bt_[:, o + col : o + col + 1]

        # ---- output-row gather: 128 rows per indirect DMA into column
        # ---- slices of one tile, one contiguous write per slice
        idx_sb = gp.tile([128, G], i32, tag="idx", name="idx_sb")
        nc.sync.dma_start(idx_sb[:], oidx[:])
        gt = gp.tile([128, G * _D], f32, tag="gath", name="gt")
        for g in range(G):
            sl = slice(g * _D, (g + 1) * _D)
            nc.gpsimd.indirect_dma_start(
                out=gt[:, sl],
                out_offset=None,
                in_=rep[:],
                in_offset=bass.IndirectOffsetOnAxis(ap=idx_sb[:, g : g + 1], axis=0),
            )
            nc.sync.dma_start(out_gath[:, sl], gt[:, sl])

        # ---- relevant-event recurrence ----
        results = []
        for l, B in enumerate(Bs) if L else []:
            B2 = 2 * B
            SD = sd_tiles[l]
            for (dst_col, slv, stile, s_col) in copies[l]:
                nc.vector.tensor_copy(
                    SD[:, dst_col : dst_col + 1],
                    results[slv][stile][:, s_col : s_col + 1],
                )
            blk = lambda op: SD[:, op * B : (op + 1) * B]
            pair = lambda op: SD[:, op * B : (op + 2) * B]
            RH, RT = blk(_OP_RH), blk(_OP_RT)
            C2, H2 = pair(_OP_CH), pair(_OP_HH)
            HH, HT = blk(_OP_HH), blk(_OP_HT)
            XHT, XHH = blk(_OP_XHT), blk(_OP_XHH)
            DT2 = pair(_OP_DTH)

            def mm4(ps, wl1, rl1, wl2, rl2, wr1, rr1, wr2, rr2):
                # left block cols [0:B], right block cols [B:2B]; each block
                # accumulates two matmuls in PSUM
                nc.tensor.matmul(ps[:, 0:B], w(wl1), rl1, start=True, stop=wl2 is None)
                if wl2 is not None:
                    nc.tensor.matmul(ps[:, 0:B], w(wl2), rl2, start=False, stop=True)
                nc.tensor.matmul(ps[:, B:B2], w(wr1), rr1, start=True, stop=wr2 is None)
                if wr2 is not None:
                    nc.tensor.matmul(ps[:, B:B2], w(wr2), rr2, start=False, stop=True)
                return ps

            def act_pair(dst, src, func, scale=1.0, bias_l=None, bias_r=None):
                # one ACT across both blocks in the zero-bias fast path,
                # else one per block with its per-partition bias
                if not has_bias or (bias_l is None and bias_r is None):
                    nc.scalar.activation(dst[:], src[:], func, scale=scale)
                else:
                    nc.scalar.activation(dst[:, 0:B], src[:, 0:B], func,
                                         bias=bias_l, scale=scale)
                    nc.scalar.activation(dst[:, B:B2], src[:, B:B2], func,
                                         bias=bias_r, scale=scale)
                return dst

            def tmp(tag):
                return tp.tile([128, B2], f32, tag=tag, name=f"t{l}_{tag}")

            # decay = exp(-w * dt)
            DEC = tmp("dec")
            nc.scalar.activation(DEC[:], DT2[:], AFT.Exp, scale=-_W_DECAY)

            # edges = tanh(rep_h @ We1 + rep_t @ We2 + be)
            EG = tmp("eg")
            ps_e = pp.tile([128, B2], f32, tag="ps", name=f"ps_e{l}")
            mm4(ps_e, "Weh1", RH, "Weh2", RT, "Wet1", RH, "Wet2", RT)
            act_pair(EG, ps_e, AFT.Tanh,
                     bias_l=bias_ap("beh") if has_bias else None,
                     bias_r=bias_ap("bet") if has_bias else None)

            # short-term memory cs = tanh(c @ Wd + bd); c_adj = c - cs + cs*dec
            CS = tmp("cs")
            ps_c = pp.tile([128, B2], f32, tag="ps", name=f"ps_c{l}")
            mm4(ps_c, "Wdh", C2[:, 0:B], None, None, "Wdt", C2[:, B:B2], None, None)
            act_pair(CS, ps_c, AFT.Tanh,
                     bias_l=bias_ap("bdh") if has_bias else None,
                     bias_r=bias_ap("bdt") if has_bias else None)
            CSD = tmp("csd")
            nc.vector.tensor_mul(CSD[:], CS[:], DEC[:])
            CMC = tmp("cmc")
            nc.vector.tensor_sub(CMC[:], C2[:], CS[:])
            CADJ = tmp("cadj")
            nc.vector.tensor_add(CADJ[:], CMC[:], CSD[:])

            # gates: z_q = edge @ Wx[:,q] + h @ Wh[:,q] + b[q]
            # sigmoid(z) computed as 0.5 + 0.5*tanh(z/2) (same ACT table set)
            gates = []
            for q in range(4):
                qs = slice(q * 128, (q + 1) * 128)
                ps_q = pp.tile([128, B2], f32, tag="ps", name=f"ps_q{l}_{q}")
                nc.tensor.matmul(ps_q[:, 0:B], w("Wxh")[:, qs], EG[:, 0:B], start=True, stop=False)
                nc.tensor.matmul(ps_q[:, 0:B], w("Whh")[:, qs], HH, start=False, stop=True)
                nc.tensor.matmul(ps_q[:, B:B2], w("Wxt")[:, qs], EG[:, B:B2], start=True, stop=False)
                nc.tensor.matmul(ps_q[:, B:B2], w("Wht")[:, qs], HT, start=False, stop=True)
                gq = tmp(f"g{q}")
                if q < 3:
                    act_pair(gq, ps_q, AFT.Tanh, scale=0.5,
                             bias_l=bias_ap("bh4h", q) if has_bias else None,
                             bias_r=bias_ap("bt4h", q) if has_bias else None)
                    import concourse.mybir as _mb
                    nc.vector.tensor_scalar(gq[:], gq[:], 0.5, 0.5,
                                            _mb.AluOpType.mult, _mb.AluOpType.add)
                else:
                    act_pair(gq, ps_q, AFT.Tanh,
                             bias_l=bias_ap("bh4", q) if has_bias else None,
                             bias_r=bias_ap("bt4", q) if has_bias else None)
                gates.append(gq)
            gi, gf, go, gg = gates

            # c_new = f*c_adj + i*g ; h_new = o*tanh(c_new)
            FC = tmp("fc")
            nc.vector.tensor_mul(FC[:], gf[:], CADJ[:])
            IG = tmp("ig")
            nc.vector.tensor_mul(IG[:], gi[:], gg[:])
            C_new = lp.tile([128, B2], f32, tag=f"res{l}_C", name=f"res{l}_C")
            nc.vector.tensor_add(C_new[:], FC[:], IG[:])
            TC = tmp("tc")
            nc.scalar.activation(TC[:], C_new[:], AFT.Tanh)
            H_new = lp.tile([128, B2], f32, tag=f"res{l}_H", name=f"res{l}_H")
            nc.vector.tensor_mul(H_new[:], go[:], TC[:])

            # combiner (uses pre-step cross hiddens XHT / XHH)
            ps_r = pp.tile([128, B2], f32, tag="ps", name=f"ps_r{l}")
            mm4(ps_r, "Wc1", H_new[:, 0:B], "Wc2", XHT,
                "Wc1", XHH, "Wc2", H_new[:, B:B2])
            R_new = lp.tile([128, B2], f32, tag=f"res{l}_R", name=f"res{l}_R")
            nc.scalar.activation(R_new[:], ps_r[:], AFT.Tanh)
            results.append({"C": C_new, "H": H_new, "R": R_new})

            o0 = sum(2 * b for b in Bs[:l])
            nc.scalar.dma_start(comp[:, o0 : o0 + B2], R_new[:])

    nc.compile()
    return nc


def _pack_weight_arrays(inputs, has_bias):
    f32 = np.float32

    def pack(names_widths, arrs):
        cols = sum(w for _, w in names_widths)
        out = np.empty((128, cols), f32)
        o = 0
        for name, wd in names_widths:
            out[:, o : o + wd] = arrs[name]
            o += wd
        return out

    arrsA = {n: np.asarray(inputs[n], f32) for n, _ in _WPACK_A}
    arrsB = {n: np.asarray(inputs[n], f32) for n, _ in _WPACK_B}
    res = {"wpackA": pack(_WPACK_A, arrsA), "wpackB": pack(_WPACK_B, arrsB)}
    if has_bias:
        bh4 = np.asarray(inputs["bh"], f32).reshape(4, 128).T
        bt4 = np.asarray(inputs["bt"], f32).reshape(4, 128).T
        arrs = {
            "beh": np.asarray(inputs["beh"], f32).reshape(128, 1),
            "bet": np.asarray(inputs["bet"], f32).reshape(128, 1),
            "bdh": np.asarray(inputs["bdh"], f32).reshape(128, 1),
            "bdt": np.asarray(inputs["bdt"], f32).reshape(128, 1),
            "bh4h": 0.5 * bh4, "bt4h": 0.5 * bt4, "bh4": bh4, "bt4": bt4,
        }
        res["bpack"] = pack(_BPACK, arrs)
    return res


def _numpy_fallback(heads, tails, times, node_rep, cell_head, hidden_head,
                    cell_tail, hidden_tail, Weh1, Weh2, beh, Wet1, Wet2, bet,
                    Wxh, Whh, bh, Wdh, bdh, Wxt, Wht, bt, Wdt, bdt, Wc1, Wc2):
    """Exact float32 reference semantics; safety net for pathological inputs."""
    f32 = np.float32
    S = heads.shape[0]
    D = node_rep.shape[1]
    rep = np.array(node_rep, f32); ch = np.array(cell_head, f32)
    hh = np.array(hidden_head, f32); ct = np.array(cell_tail, f32)
    ht = np.array(hidden_tail, f32)
    rt = np.zeros(node_rep.shape[0], f32)
    out = np.zeros((2, S, D), f32)

    def sig(x):
        return f32(1.0) / (f32(1.0) + np.exp(-x, dtype=f32))

    def tlstm(x, c, h, dec, Wx, Wh, b, Wd, bd):
        cs = np.tanh(c @ Wd + bd, dtype=f32)
        c_adj = c - cs + cs * dec
        z = x @ Wx + h @ Wh + b
        i, f, o, g = np.split(z, 4)
        i = sig(i); f = sig(f); o = sig(o); g = np.tanh(g, dtype=f32)
        c_new = f * c_adj + i * g
        return c_new, o * np.tanh(c_new, dtype=f32)

    for j in range(S):
        h_i = int(heads[j]); t_i = int(tails[j]); tm = f32(times[j])
        rep_h = rep[h_i].copy(); rep_t = rep[t_i].copy()
        out[0, j] = rep_h; out[1, j] = rep_t
        dec_h = np.exp(f32(-_W_DECAY) * (tm - rt[h_i]), dtype=f32)
        dec_t = np.exp(f32(-_W_DECAY) * (tm - rt[t_i]), dtype=f32)
        edge_h = np.tanh(rep_h @ Weh1 + rep_t @ Weh2 + beh, dtype=f32)
        edge_t = np.tanh(rep_h @ Wet1 + rep_t @ Wet2 + bet, dtype=f32)
        c_h, h_h = tlstm(edge_h, ch[h_i], hh[h_i], dec_h, Wxh, Whh, bh, Wdh, bdh)
        c_t, h_t = tlstm(edge_t, ct[t_i], ht[t_i], dec_t, Wxt, Wht, bt, Wdt, bdt)
        new_rep_h = np.tanh(h_h @ Wc1 + ht[h_i] @ Wc2, dtype=f32)
        new_rep_t = np.tanh(hh[t_i] @ Wc1 + h_t @ Wc2, dtype=f32)
        rep[h_i] = new_rep_h; rep[t_i] = new_rep_t
        ch[h_i] = c_h; hh[h_i] = h_h; ct[t_i] = c_t; ht[t_i] = h_t
        rt[h_i] = tm; rt[t_i] = tm
    return out


def kernel(**inputs):
    global last_result
    heads = np.asarray(inputs["heads"]).astype(np.int64)
    tails = np.asarray(inputs["tails"]).astype(np.int64)
    times = np.asarray(inputs["times"], dtype=np.float32)
    node_rep = np.asarray(inputs["node_rep"], dtype=np.float32)
    N = node_rep.shape[0]
    S = heads.shape[0]

    plan = _preprocess(heads, tails, times)
    if plan is None:
        return _numpy_fallback(
            heads, tails, times,
            *[np.asarray(inputs[k], np.float32) for k in (
                "node_rep", "cell_head", "hidden_head", "cell_tail", "hidden_tail",
                "Weh1", "Weh2", "beh", "Wet1", "Wet2", "bet",
                "Wxh", "Whh", "bh", "Wdh", "bdh", "Wxt", "Wht", "bt", "Wdt", "bdt",
                "Wc1", "Wc2")],
        )

    routing = _route_outputs(heads, tails, plan, N)
    shard, G, n_pad = routing["shard"], routing["G"], routing["n_pad"]
    Bs, Ctot = plan["Bs"], plan["Ctot"]
    L = len(Bs)

    has_bias = bool(L) and any(
        np.any(np.asarray(inputs[k], np.float32))
        for k in ("beh", "bet", "bdh", "bdt", "bh", "bt"))

    sig = (shard, G, tuple(Bs),
           tuple(tuple(c) for lc in plan["copies"] for c in lc), Ctot, has_bias)
    nc = _cache.get(sig)
    if nc is None:
        nc = _build_program(shard, G, Bs, plan["copies"], Ctot, has_bias)
        _cache[sig] = nc

    # per-level packed seed tensors [128, 10*B] from the state tables
    tables = {k: np.asarray(inputs[k], np.float32) for k in (
        "node_rep", "cell_head", "hidden_head", "cell_tail", "hidden_tail")}
    seeds = [np.zeros((128, _N_OPS * B), np.float32) for B in Bs]
    for l, B in enumerate(Bs):
        for (col, tab, node) in plan["seed_fill"][l]:
            seeds[l][:, col] = tables[tab][node]
        for (p, dt_h, dt_t) in plan["dt_fill"][l]:
            seeds[l][:, _OP_DTH * B + p] = dt_h
            seeds[l][:, _OP_DTT * B + p] = dt_t

    w_in = _pack_weight_arrays(inputs, has_bias) if L else {}

    pad_rows = shard * _NCORES - N
    rep_padded = node_rep if pad_rows == 0 else np.vstack(
        [node_rep, np.zeros((pad_rows, _D), np.float32)])
    in_maps = []
    for k in range(_NCORES):
        m = {
            "rep": np.ascontiguousarray(rep_padded[k * shard : (k + 1) * shard]),
            "oidx": routing["oidx"][k],
        }
        if L:
            m.update(w_in)
            for l in range(L):
                m[f"seeds{l}"] = seeds[l]
        in_maps.append(m)

    from concourse import bass_utils
    res = bass_utils.run_bass_kernel_spmd(nc, in_maps, core_ids=list(range(_NCORES)))
    last_result = res

    out_flat = np.zeros((2 * S, _D), np.float32)
    for k in range(_NCORES):
        slots = routing["gl_slot"][k]
        if slots:
            rows = res.results[k]["out_gath"].reshape(128, G, _D)
            rows = rows.transpose(1, 0, 2).reshape(n_pad, _D)
            out_flat[np.asarray(slots)] = rows[: len(slots)]
    if len(routing["comp_slots"]):
        comp_v = res.results[0]["comp"]
        out_flat[routing["comp_slots"]] = comp_v[:, routing["comp_cols"]].T
    return out_flat.reshape(2, S, _D)
